# revision 6
# baseline (speedup 1.0000x reference)
"""Trainium2 kernel: embedding sum-pool via chunked dma_gather + PE matmul.

Per core (1024 batches, 20 lookups each = 20480 rows from a 1M x 128 table):
  1. Host buckets lookups by 32768-row table chunk (int16-addressable),
     sorts by batch within each chunk, pads each chunk to whole slices of
     128 rows (pad entries repeat a real index; their SEL columns are 0).
  2. 31 dma_gather instructions (one per chunk) spread over 4 SWDGE queues
     pull rows (bf16, 256B) into SBUF; descriptor generation runs in
     parallel ucode queue contexts.
  3. Each gathered slice of 128 rows is routed to its batch rows by a
     one-hot selection matmul on the PE, accumulating the 8 batch-blocks
     in PSUM (exact f32 accumulation).
  4. PSUM -> SBUF -> DRAM. Host concatenates per-core outputs.

The program is JIT-specialized per call: slice counts and the matmul plan
are baked in (common across cores so one SPMD program serves all 8).
"""

import numpy as np

import concourse.bacc as bacc
import concourse.bass as bass
import concourse.tile as tile
from concourse import mybir

V = 1_000_000
D = 128
L = 20
P = 128
BC = 1024          # batches per core
NBLK = BC // P     # 8 batch blocks of 128
CH = 32768         # chunk rows (int16-addressable)
NCH = (V + CH - 1) // CH   # 31 chunks
N_CORES = 8
NQ = 4             # SWDGE queues


def wrap16(arr):
    """dma_gather idx layout: [128, n//16] int16, idx i at (i%16, i//16),
    replicated across the 8 16-partition groups."""
    s = arr.shape[0] // 16
    out = np.empty((P, s), dtype=np.int16)
    base = arr.reshape(s, 16).T
    for g in range(8):
        out[g * 16 : (g + 1) * 16] = base
    return out


def sel_for_slice(bs):
    """One-hot SELs for a slice's batch vector (pads marked b=-1)."""
    out = {}
    valid = bs >= 0
    for B in np.unique(bs[valid] // P):
        m = np.zeros((P, P), dtype=np.float32)
        rows = np.nonzero(valid & (bs // P == B))[0]
        m[rows, bs[rows] % P] = 1.0
        out[int(B)] = m
    return out


def plan_core(flat_idx, flat_b, slices_per_chunk, n_ind_slices):
    """Full 128-row slices per chunk go to dma_gather; each chunk's
    leftover (n mod 128) plus `offload_per_chunk` additional full slices
    go to int32 indirect1d slices (chunk-free).

    Returns (idx16 [gather positions], idx32 [P, n_ind_slices],
             meta2sel {(slice,B): sel}).  Slice numbering: gather slices
    chunk-major first, then indirect slices.
    """
    total_pos = sum(slices_per_chunk) * P
    idx16 = np.zeros(total_pos, dtype=np.int16)
    meta2sel = {}
    leftover_i = []   # full-table int32 indices
    leftover_b = []
    pos0 = 0
    base_slice = 0
    for c in range(NCH):
        ns = slices_per_chunk[c]
        mask = (flat_idx >> 15) == c
        sub_f = flat_idx[mask]
        sub_b = flat_b[mask]
        order = np.argsort(sub_b, kind="stable")
        sub_f, sub_b = sub_f[order], sub_b[order]
        n = sub_f.shape[0]
        ng = min(ns * P, n)
        # gather part
        cap = ns * P
        gi = (sub_f[:ng] & (CH - 1)).astype(np.int16)
        pad = np.full(cap, gi[0] if ng else 0, dtype=np.int16)
        pad[:ng] = gi
        idx16[pos0 : pos0 + cap] = pad
        gb = np.full(cap, -1, dtype=np.int64)
        gb[:ng] = sub_b[:ng]
        for k in range(ns):
            bs = gb[k * P : (k + 1) * P]
            for B, m in sel_for_slice(bs).items():
                meta2sel[(base_slice + k, B)] = m
        # leftover part
        leftover_i.append(sub_f[ng:])
        leftover_b.append(sub_b[ng:])
        pos0 += cap
        base_slice += ns
    li = np.concatenate(leftover_i)
    lb = np.concatenate(leftover_b)
    order = np.argsort(lb, kind="stable")
    li, lb = li[order], lb[order]
    n = li.shape[0]
    cap = n_ind_slices * P
    assert n <= cap, (n, cap)
    pi = np.zeros(cap, dtype=np.int32)
    pi[:n] = li.astype(np.int32)
    pb = np.full(cap, -1, dtype=np.int64)
    pb[:n] = lb
    for k in range(n_ind_slices):
        bs = pb[k * P : (k + 1) * P]
        for B, m in sel_for_slice(bs).items():
            meta2sel[(base_slice + k, B)] = m
    # indirect idx layout: slice k -> idx32[:, k] (one per partition)
    idx32 = np.ascontiguousarray(
        pi.reshape(n_ind_slices, P).T.astype(np.int32)
    )
    return idx16, idx32, meta2sel


def build_full(slices_per_chunk, n_ind_slices, mm_plan):
    """Build the SPMD program with the matmul plan baked in.

    mm_plan: sorted list of (slice, B); slices are gather-chunk-major
    followed by n_ind_slices indirect slices.
    """
    total_slices = sum(slices_per_chunk)
    n_mm = len(mm_plan)
    nc = bacc.Bacc(
        "TRN2",
        target_bir_lowering=False,
        debug=False,
        dynamic_dma_scratch_size=32768,
        num_swdge_queues=NQ,
    )
    table = nc.dram_tensor("table", [V, D], mybir.dt.bfloat16,
                           kind="ExternalInput")
    idx = nc.dram_tensor("idx", [P, (total_slices * P) // 16],
                         mybir.dt.int16, kind="ExternalInput")
    idx32 = nc.dram_tensor("idx32", [P, n_ind_slices], mybir.dt.int32,
                           kind="ExternalInput")
    selt = nc.dram_tensor("sel", [P, n_mm * P], mybir.dt.bfloat16,
                          kind="ExternalInput")
    out = nc.dram_tensor("out", [BC, D], mybir.dt.float32,
                         kind="ExternalOutput")

    # slice -> (chunk, offset-within-chunk); indirect slices get
    # pseudo-chunks of 4 slices for SEL grouping
    slice_chunk, slice_off = [], []
    for c in range(NCH):
        for k in range(slices_per_chunk[c]):
            slice_chunk.append(c)
            slice_off.append(k)
    n_gs = len(slice_chunk)
    for k in range(n_ind_slices):
        slice_chunk.append(NCH + k // 4)
        slice_off.append(k)

    # chunk -> contiguous matmul index range (plan is slice-major sorted)
    mm_lo = {}
    mm_hi = {}
    for i, (s, B) in enumerate(mm_plan):
        c = slice_chunk[s]
        mm_lo.setdefault(c, i)
        mm_hi[c] = i + 1

    first_mm, last_mm = {}, {}
    for i, (s, B) in enumerate(mm_plan):
        first_mm.setdefault(B, i)
        last_mm[B] = i

    with tile.TileContext(nc) as tc:
        with (
            tc.tile_pool(name="io", bufs=1) as io_pool,
            tc.tile_pool(name="g", bufs=NCH) as gpool,
            tc.tile_pool(name="ps", bufs=1, space="PSUM") as pspool,
        ):
            idx_t = io_pool.tile([P, (total_slices * P) // 16],
                                 mybir.dt.int16)
            nc.sync.dma_start(idx_t[:], idx[:, :])
            idx32_t = io_pool.tile([P, n_ind_slices], mybir.dt.int32)
            nc.sync.dma_start(idx32_t[:], idx32[:, :])
            psum_banks = [
                pspool.tile([P, P], mybir.dt.float32, name=f"psumbank{i}")
                for i in range(NBLK)
            ]

            def psum_slice(B):
                return psum_banks[B][:, :]
            pooled = io_pool.tile([P, NBLK * D], mybir.dt.float32)

            # indirect (int32) slices first: queue-0 ops at Pool-DMA
            # positions 0..n_ind-1 keep the DMASW lane pattern legal when
            # gather queue_num is offset by n_ind below
            gi_tiles = []
            for k in range(n_ind_slices):
                git = gpool.tile([P, D], mybir.dt.bfloat16, name="git")
                gi_tiles.append(git)
                nc.gpsimd.indirect_dma_start(
                    out=git[:, :],
                    out_offset=None,
                    in_=table[:, :],
                    in_offset=bass.IndirectOffsetOnAxis(
                        ap=idx32_t[:, k : k + 1], axis=0
                    ),
                )

            # all gathers (round-robin queues, offset past the indirects)
            g_tiles = []
            pos0 = 0
            for c in range(NCH):
                ns = slices_per_chunk[c]
                if ns == 0:
                    g_tiles.append(None)
                    continue
                npos = ns * P
                rows_c = min(CH, V - c * CH)
                gt = gpool.tile([P, ns * D], mybir.dt.bfloat16)
                g_tiles.append(gt)
                nc.gpsimd.dma_gather(
                    out_ap=gt[:, :].rearrange("p (g e) -> p g e", g=ns),
                    in_ap=table[c * CH : c * CH + rows_c, :],
                    idxs_ap=idx_t[:, pos0 // 16 : (pos0 + npos) // 16],
                    num_idxs=npos,
                    num_idxs_reg=npos,
                    elem_size=D,
                    single_packet=False,
                    queue_num=(n_ind_slices + c) % NQ,
                )
                pos0 += npos

            def slice_rhs(s):
                if s < n_gs:
                    c = slice_chunk[s]
                    k = slice_off[s]
                    return g_tiles[c][:, k * D : (k + 1) * D]
                return gi_tiles[s - n_gs][:, :]

            # per-chunk SEL stream (alternating HWDGE rings) + matmuls
            mx_mm = max(
                mm_hi[c] - mm_lo[c] for c in range(NCH) if c in mm_lo
            )
            with tc.tile_pool(name="selp", bufs=8) as spool:
                for c in range(NCH + (n_ind_slices + 3) // 4):
                    if c not in mm_lo:
                        continue
                    lo, hi = mm_lo[c], mm_hi[c]
                    st = spool.tile([P, mx_mm * P], mybir.dt.bfloat16,
                                    name="st")
                    eng = nc.sync if c % 2 == 0 else nc.scalar
                    eng.dma_start(st[:, : (hi - lo) * P],
                                  selt[:, lo * P : hi * P])
                    for i in range(lo, hi):
                        s, B = mm_plan[i]
                        nc.tensor.matmul(
                            out=psum_slice(B),
                            lhsT=st[:, (i - lo) * P : (i - lo + 1) * P],
                            rhs=slice_rhs(s),
                            start=(first_mm[B] == i),
                            stop=(last_mm[B] == i),
                        )

            out_v = out.rearrange("(B p) e -> p B e", p=P)
            for B in range(NBLK):
                nc.vector.tensor_copy(
                    out=pooled[:, B * D : (B + 1) * D], in_=psum_slice(B)
                )
                nc.sync.dma_start(
                    out_v[:, B : B + 1, :],
                    pooled[:, B * D : (B + 1) * D].rearrange(
                        "p (o e) -> p o e", o=1
                    ),
                )
    nc.compile()
    return nc


LAST_RESULT = None


def _ensure_axon_hooks():
    import sys
    import types

    if "antenv.axon_hooks" in sys.modules:
        return
    try:
        import antenv
    except ImportError:
        return
    if hasattr(antenv, "axon_hooks"):
        sys.modules.setdefault("antenv.axon_hooks", antenv.axon_hooks)
        return
    mod = types.ModuleType("antenv.axon_hooks")
    holder = [None]
    mod.set_axon_ntff_profile_hook = lambda h: holder.__setitem__(0, h)
    mod.get_axon_ntff_profile_hook = lambda: holder[0]
    sys.modules["antenv.axon_hooks"] = mod
    antenv.axon_hooks = mod


def prepare(indices):
    """Host planning for all 8 cores -> common plan + per-core feeds."""
    indices = np.asarray(indices)
    per_core = []
    for core in range(N_CORES):
        f = 0 if core < 4 else 1
        blk = core % 4
        sub = indices[f, blk * BC : (blk + 1) * BC, :].astype(np.int64)
        flat_idx = sub.reshape(-1)
        flat_b = np.repeat(np.arange(BC, dtype=np.int64), L)
        per_core.append((flat_idx, flat_b))

    counts = np.zeros((N_CORES, NCH), dtype=np.int64)
    for core, (fi, _) in enumerate(per_core):
        counts[core] = np.bincount(fi >> 15, minlength=NCH)

    # full gather slices per chunk, then offload OFF_K chunks by one
    # slice each to the Pool-inline indirect path to balance engines
    OFF_K = 0
    base = [-(-int(counts[:, c].max()) // P) for c in range(NCH)]
    slices_per_chunk = [max(1, base[c]) for c in range(NCH)]
    # leftovers per core -> common indirect slice count
    n_ind = 0
    for core in range(N_CORES):
        ng = sum(
            min(slices_per_chunk[c] * P, int(counts[core, c]))
            for c in range(NCH)
        )
        n_ind = max(n_ind, -(-(BC * L - ng) // P))
    n_ind_slices = max(1, n_ind)

    plans = [plan_core(fi, fb, slices_per_chunk, n_ind_slices)
             for fi, fb in per_core]

    mm_plan = sorted({m for _, _, meta in plans for m in meta})
    mm_index = {m: i for i, m in enumerate(mm_plan)}
    n_mm = len(mm_plan)

    feeds = []
    for idx16, idx32, meta2sel in plans:
        sel_full = np.zeros((n_mm, P, P), dtype=np.float32)
        for m, mat in meta2sel.items():
            sel_full[mm_index[m]] = mat
        sel_host = np.ascontiguousarray(
            sel_full.transpose(1, 0, 2).reshape(P, n_mm * P)
        )
        feeds.append({"idx": wrap16(idx16), "idx32": idx32,
                      "sel": sel_host})
    return slices_per_chunk, n_ind_slices, mm_plan, feeds


def kernel(indices, table0, table1):
    from concourse.bass_utils import run_bass_kernel_spmd
    import ml_dtypes

    _ensure_axon_hooks()
    global LAST_RESULT

    slices_per_chunk, n_ind_slices, mm_plan, feeds = prepare(indices)
    nc = build_full(slices_per_chunk, n_ind_slices, mm_plan)

    t0 = np.asarray(np.asarray(table0), dtype=ml_dtypes.bfloat16)
    t1 = np.asarray(np.asarray(table1), dtype=ml_dtypes.bfloat16)
    in_maps = []
    for core in range(N_CORES):
        fd = dict(feeds[core])
        fd["table"] = t0 if core < 4 else t1
        fd["sel"] = fd["sel"].astype(ml_dtypes.bfloat16)
        in_maps.append(fd)

    LAST_RESULT = run_bass_kernel_spmd(nc, in_maps,
                                       core_ids=list(range(N_CORES)))
    outs = [r["out"] for r in LAST_RESULT.results]
    pooled0 = np.concatenate(outs[0:4], axis=0)
    pooled1 = np.concatenate(outs[4:8], axis=0)
    return np.concatenate([pooled0, pooled1], axis=1).astype(np.float32)


# revision 7
# speedup vs baseline: 1.0964x; 1.0964x over previous
"""Trainium2 kernel: embedding sum-pool via chunked dma_gather + PE matmul.

Per core (1024 batches, 20 lookups each = 20480 rows from a 1M x 128 table):
  1. Host buckets lookups by 32768-row table chunk (int16-addressable),
     sorts by batch within each chunk, pads each chunk to whole slices of
     128 rows (pad entries repeat a real index; their SEL columns are 0).
  2. 31 dma_gather instructions (one per chunk) spread over 4 SWDGE queues
     pull rows (bf16, 256B) into SBUF; descriptor generation runs in
     parallel ucode queue contexts.
  3. Each gathered slice of 128 rows is routed to its batch rows by a
     one-hot selection matmul on the PE, accumulating the 8 batch-blocks
     in PSUM (exact f32 accumulation).
  4. PSUM -> SBUF -> DRAM. Host concatenates per-core outputs.

The program is JIT-specialized per call: slice counts and the matmul plan
are baked in (common across cores so one SPMD program serves all 8).
"""

import numpy as np

import concourse.bacc as bacc
import concourse.bass as bass
import concourse.tile as tile
from concourse import mybir

V = 1_000_000
D = 128
L = 20
P = 128
BC = 1024          # batches per core
NBLK = BC // P     # 8 batch blocks of 128
CH = 32768         # chunk rows (int16-addressable)
NCH = (V + CH - 1) // CH   # 31 chunks
N_CORES = 8
NQ = 4             # SWDGE queues


def wrap16(arr):
    """dma_gather idx layout: [128, n//16] int16, idx i at (i%16, i//16),
    replicated across the 8 16-partition groups."""
    s = arr.shape[0] // 16
    out = np.empty((P, s), dtype=np.int16)
    base = arr.reshape(s, 16).T
    for g in range(8):
        out[g * 16 : (g + 1) * 16] = base
    return out


def sel_for_slice(bs):
    """One-hot SELs for a slice's batch vector (pads marked b=-1)."""
    out = {}
    valid = bs >= 0
    for B in np.unique(bs[valid] // P):
        m = np.zeros((P, P), dtype=np.float32)
        rows = np.nonzero(valid & (bs // P == B))[0]
        m[rows, bs[rows] % P] = 1.0
        out[int(B)] = m
    return out


def plan_core(flat_idx, flat_b, slices_per_chunk, n_ind_slices):
    """Full 128-row slices per chunk go to dma_gather; each chunk's
    leftover (n mod 128) plus `offload_per_chunk` additional full slices
    go to int32 indirect1d slices (chunk-free).

    Returns (idx16 [gather positions], idx32 [P, n_ind_slices],
             meta2sel {(slice,B): sel}).  Slice numbering: gather slices
    chunk-major first, then indirect slices.
    """
    total_pos = sum(slices_per_chunk) * P
    idx16 = np.zeros(total_pos, dtype=np.int16)
    meta2sel = {}
    leftover_i = []   # full-table int32 indices
    leftover_b = []
    pos0 = 0
    base_slice = 0
    for c in range(NCH):
        ns = slices_per_chunk[c]
        mask = (flat_idx >> 15) == c
        sub_f = flat_idx[mask]
        sub_b = flat_b[mask]
        order = np.argsort(sub_b, kind="stable")
        sub_f, sub_b = sub_f[order], sub_b[order]
        n = sub_f.shape[0]
        ng = min(ns * P, n)
        # gather part
        cap = ns * P
        gi = (sub_f[:ng] & (CH - 1)).astype(np.int16)
        pad = np.full(cap, gi[0] if ng else 0, dtype=np.int16)
        pad[:ng] = gi
        idx16[pos0 : pos0 + cap] = pad
        gb = np.full(cap, -1, dtype=np.int64)
        gb[:ng] = sub_b[:ng]
        for k in range(ns):
            bs = gb[k * P : (k + 1) * P]
            for B, m in sel_for_slice(bs).items():
                meta2sel[(base_slice + k, B)] = m
        # leftover part
        leftover_i.append(sub_f[ng:])
        leftover_b.append(sub_b[ng:])
        pos0 += cap
        base_slice += ns
    li = np.concatenate(leftover_i)
    lb = np.concatenate(leftover_b)
    order = np.argsort(lb, kind="stable")
    li, lb = li[order], lb[order]
    n = li.shape[0]
    cap = n_ind_slices * P
    assert n <= cap, (n, cap)
    pi = np.zeros(cap, dtype=np.int32)
    pi[:n] = li.astype(np.int32)
    pb = np.full(cap, -1, dtype=np.int64)
    pb[:n] = lb
    for k in range(n_ind_slices):
        bs = pb[k * P : (k + 1) * P]
        for B, m in sel_for_slice(bs).items():
            meta2sel[(base_slice + k, B)] = m
    # indirect idx layout: slice k -> idx32[:, k] (one per partition)
    idx32 = np.ascontiguousarray(
        pi.reshape(n_ind_slices, P).T.astype(np.int32)
    )
    return idx16, idx32, meta2sel


def build_full(slices_per_chunk, n_ind_slices, mm_plan):
    """Build the SPMD program with the matmul plan baked in.

    mm_plan: sorted list of (slice, B); slices are gather-chunk-major
    followed by n_ind_slices indirect slices.
    """
    total_slices = sum(slices_per_chunk)
    n_mm = len(mm_plan)
    nc = bacc.Bacc(
        "TRN2",
        target_bir_lowering=False,
        debug=False,
        dynamic_dma_scratch_size=32768,
        num_swdge_queues=NQ,
    )
    table = nc.dram_tensor("table", [V, D], mybir.dt.bfloat16,
                           kind="ExternalInput")
    idx = nc.dram_tensor("idx", [P, (total_slices * P) // 16],
                         mybir.dt.int16, kind="ExternalInput")

    selt = nc.dram_tensor("sel", [P, n_mm * P], mybir.dt.bfloat16,
                          kind="ExternalInput")
    out = nc.dram_tensor("out", [BC, D], mybir.dt.float32,
                         kind="ExternalOutput")

    # slice -> (chunk, offset-within-chunk); indirect slices get
    # pseudo-chunks of 4 slices for SEL grouping
    slice_chunk, slice_off = [], []
    for c in range(NCH):
        for k in range(slices_per_chunk[c]):
            slice_chunk.append(c)
            slice_off.append(k)
    n_gs = len(slice_chunk)
    for k in range(n_ind_slices):
        slice_chunk.append(NCH + k // 4)
        slice_off.append(k)

    # chunk -> contiguous matmul index range (plan is slice-major sorted)
    mm_lo = {}
    mm_hi = {}
    for i, (s, B) in enumerate(mm_plan):
        c = slice_chunk[s]
        mm_lo.setdefault(c, i)
        mm_hi[c] = i + 1

    first_mm, last_mm = {}, {}
    for i, (s, B) in enumerate(mm_plan):
        first_mm.setdefault(B, i)
        last_mm[B] = i

    with tile.TileContext(nc) as tc:
        with (
            tc.tile_pool(name="io", bufs=1) as io_pool,
            tc.tile_pool(name="g", bufs=NCH) as gpool,
            tc.tile_pool(name="ps", bufs=1, space="PSUM") as pspool,
        ):
            idx_t = io_pool.tile([P, (total_slices * P) // 16],
                                 mybir.dt.int16)
            nc.sync.dma_start(idx_t[:], idx[:, :])

            psum_banks = [
                pspool.tile([P, P], mybir.dt.float32, name=f"psumbank{i}")
                for i in range(NBLK)
            ]

            def psum_slice(B):
                return psum_banks[B][:, :]
            pooled = io_pool.tile([P, NBLK * D], mybir.dt.float32)

            # all gathers (round-robin queues)
            g_tiles = []
            pos0 = 0
            for c in range(NCH):
                ns = slices_per_chunk[c]
                if ns == 0:
                    g_tiles.append(None)
                    continue
                npos = ns * P
                rows_c = min(CH, V - c * CH)
                gt = gpool.tile([P, ns * D], mybir.dt.bfloat16)
                g_tiles.append(gt)
                nc.gpsimd.dma_gather(
                    out_ap=gt[:, :].rearrange("p (g e) -> p g e", g=ns),
                    in_ap=table[c * CH : c * CH + rows_c, :],
                    idxs_ap=idx_t[:, pos0 // 16 : (pos0 + npos) // 16],
                    num_idxs=npos,
                    num_idxs_reg=npos,
                    elem_size=D,
                    single_packet=False,
                    queue_num=c % NQ,
                )
                pos0 += npos

            def slice_rhs(s):
                c = slice_chunk[s]
                k = slice_off[s]
                return g_tiles[c][:, k * D : (k + 1) * D]

            # per-chunk SEL stream (alternating HWDGE rings) + matmuls
            mx_mm = max(
                mm_hi[c] - mm_lo[c] for c in range(NCH) if c in mm_lo
            )
            with tc.tile_pool(name="selp", bufs=8) as spool:
                for c in range(NCH + (n_ind_slices + 3) // 4):
                    if c not in mm_lo:
                        continue
                    lo, hi = mm_lo[c], mm_hi[c]
                    st = spool.tile([P, mx_mm * P], mybir.dt.bfloat16,
                                    name="st")
                    eng = nc.sync if c % 2 == 0 else nc.scalar
                    eng.dma_start(st[:, : (hi - lo) * P],
                                  selt[:, lo * P : hi * P])
                    for i in range(lo, hi):
                        s, B = mm_plan[i]
                        nc.tensor.matmul(
                            out=psum_slice(B),
                            lhsT=st[:, (i - lo) * P : (i - lo + 1) * P],
                            rhs=slice_rhs(s),
                            start=(first_mm[B] == i),
                            stop=(last_mm[B] == i),
                        )

            out_v = out.rearrange("(B p) e -> p B e", p=P)
            for B in range(NBLK):
                nc.vector.tensor_copy(
                    out=pooled[:, B * D : (B + 1) * D], in_=psum_slice(B)
                )
                nc.sync.dma_start(
                    out_v[:, B : B + 1, :],
                    pooled[:, B * D : (B + 1) * D].rearrange(
                        "p (o e) -> p o e", o=1
                    ),
                )
    nc.compile()
    return nc


LAST_RESULT = None


def _ensure_axon_hooks():
    import sys
    import types

    if "antenv.axon_hooks" in sys.modules:
        return
    try:
        import antenv
    except ImportError:
        return
    if hasattr(antenv, "axon_hooks"):
        sys.modules.setdefault("antenv.axon_hooks", antenv.axon_hooks)
        return
    mod = types.ModuleType("antenv.axon_hooks")
    holder = [None]
    mod.set_axon_ntff_profile_hook = lambda h: holder.__setitem__(0, h)
    mod.get_axon_ntff_profile_hook = lambda: holder[0]
    sys.modules["antenv.axon_hooks"] = mod
    antenv.axon_hooks = mod


def prepare(indices):
    """Host planning for all 8 cores -> common plan + per-core feeds."""
    indices = np.asarray(indices)
    per_core = []
    for core in range(N_CORES):
        f = 0 if core < 4 else 1
        blk = core % 4
        sub = indices[f, blk * BC : (blk + 1) * BC, :].astype(np.int64)
        flat_idx = sub.reshape(-1)
        flat_b = np.repeat(np.arange(BC, dtype=np.int64), L)
        per_core.append((flat_idx, flat_b))

    counts = np.zeros((N_CORES, NCH), dtype=np.int64)
    for core, (fi, _) in enumerate(per_core):
        counts[core] = np.bincount(fi >> 15, minlength=NCH)

    # full gather slices per chunk, then offload OFF_K chunks by one
    # slice each to the Pool-inline indirect path to balance engines
    OFF_K = 0
    base = [-(-int(counts[:, c].max()) // P) for c in range(NCH)]
    slices_per_chunk = [max(1, base[c]) for c in range(NCH)]
    # leftovers per core -> common indirect slice count
    n_ind_slices = 0

    plans = [plan_core(fi, fb, slices_per_chunk, n_ind_slices)
             for fi, fb in per_core]

    mm_plan = sorted({m for _, _, meta in plans for m in meta})
    mm_index = {m: i for i, m in enumerate(mm_plan)}
    n_mm = len(mm_plan)

    feeds = []
    for idx16, idx32, meta2sel in plans:
        sel_full = np.zeros((n_mm, P, P), dtype=np.float32)
        for m, mat in meta2sel.items():
            sel_full[mm_index[m]] = mat
        sel_host = np.ascontiguousarray(
            sel_full.transpose(1, 0, 2).reshape(P, n_mm * P)
        )
        feeds.append({"idx": wrap16(idx16), "sel": sel_host})
    return slices_per_chunk, n_ind_slices, mm_plan, feeds


def kernel(indices, table0, table1):
    from concourse.bass_utils import run_bass_kernel_spmd
    import ml_dtypes

    _ensure_axon_hooks()
    global LAST_RESULT

    slices_per_chunk, n_ind_slices, mm_plan, feeds = prepare(indices)
    nc = build_full(slices_per_chunk, n_ind_slices, mm_plan)

    t0 = np.asarray(np.asarray(table0), dtype=ml_dtypes.bfloat16)
    t1 = np.asarray(np.asarray(table1), dtype=ml_dtypes.bfloat16)
    in_maps = []
    for core in range(N_CORES):
        fd = dict(feeds[core])
        fd["table"] = t0 if core < 4 else t1
        fd["sel"] = fd["sel"].astype(ml_dtypes.bfloat16)
        in_maps.append(fd)

    LAST_RESULT = run_bass_kernel_spmd(nc, in_maps,
                                       core_ids=list(range(N_CORES)))
    outs = [r["out"] for r in LAST_RESULT.results]
    pooled0 = np.concatenate(outs[0:4], axis=0)
    pooled1 = np.concatenate(outs[4:8], axis=0)
    return np.concatenate([pooled0, pooled1], axis=1).astype(np.float32)
